# revision 27
# baseline (speedup 1.0000x reference)
"""Trainium2 Bass kernel for BertSelfAttentionDistance.

Problem (per batch b, B=8, S=2048, H=1024, NH=1, DT=64):
    q = hs @ Wq.T + bq ; k = hs @ Wk.T + bk ; v = hs @ Wv.T + bv
    scores = q @ k.T
    wdd    = q @ dist_emb.T                       # [S, DT]
    bias   = take_along(wdd, rel, axis=-1) * (rel == 1)
    out    = softmax((scores + bias)/sqrt(H) + attn_mask) @ v

Key simplifications:
1. Because the gathered value is multiplied by (rel == 1), only
   wdd[:, 1] = q @ dist_emb[1] survives:
       bias[i, j] = (rel[i, j] == 1) * (q[i] . dist_emb[1])
   so the [S, S] gather is never needed — just a compare and broadcast.
2. q and k are never needed individually — only scores and w1:
       scores0 = hs @ (Wq.T @ Wk) @ hs.T = G @ hs.T,   G = hs @ M
       w1      = hs @ (Wq.T @ dist_emb[1]) (+ bq . dist_emb[1])
   M = Wq.T @ Wk and m1 = Wq.T @ d1 are folded on the host (weight-only,
   O(H^2) preprocessing); this removes one full [S,H]x[H,H] projection
   per core and the hs tile doubles as the scores stationary.
   Bias terms: q0.bk and bq.bk are constant per query row -> softmax-
   invariant, dropped exactly. bq.k0[k] varies per key: handled by a
   conditionally-compiled kb path (has_bq) feeding the exp bias; the
   benchmark has bq = 0 so the lean variant is used.

Sharding: pure data-parallel, one batch per NeuronCore (8 batches, 8 cores).

Per-core layout (ST = transposed-scores layout, keys on partitions):
    GT[d, s]  (via M stationary / hsT moving; split GTb bf16 / G8 fp8e4)
    ST[j, q] = sum_d hsT[d, j] * GT[d, q]         (keys j on partitions)
    E[j, q]  = exp(ST/32 + (relT==1)*w1[q]/32 + am[j])   (unnormalized)
    denom[q] = ones[1,j] @ E    (PE reduction over partitions, 4x col-tiled)
    ctxT[d, q] = (sum_j (V[j, d]+bv[d]) * E[j, q]) * (1/denom[q])

Mixed precision: the scores contraction runs 2 of 8 k-tiles in bf16 and
6 as fp8e4 DoubleRow pairs (2x PE throughput).  Measured end-to-end
max-rel error 1.63e-2 of output scale (deterministic for the benchmark
inputs) vs the 2e-2 gate; all other matmuls stay bf16 (fp8 anywhere
else pushes past the gate - each further tensor quantization adds
~2e-2 in quadrature).

v bias note: softmax rows sum to 1, so ctx = P @ (V0 + 1*bv) = P@V0 + bv;
with unnormalized E: (E@(V0+1*bv)) / denom = ctx0 + bv exactly.
"""

import sys

sys.path.insert(0, "/opt/trn_rl_repo")

from contextlib import ExitStack

import ml_dtypes
import numpy as np

import concourse.bass as bass
import concourse.tile as tile
from concourse import bacc, mybir
from concourse._compat import with_exitstack
from concourse.bass_utils import run_bass_kernel_spmd

B, S, H, DT = 8, 2048, 1024, 64
NCORES = 8
P = 128
CHUNK = 512  # q-chunk = one fp32 PSUM bank
SCALE = 1.0 / 32.0  # 1/sqrt(H)
NDUMMY = 26

BF16 = mybir.dt.bfloat16
F8 = mybir.dt.float8e4
F32 = mybir.dt.float32
I32 = mybir.dt.int32
AF = mybir.ActivationFunctionType
ALU = mybir.AluOpType
DR = mybir.MatmulPerfMode.DoubleRow
# k-tiles of the scores contraction computed in bf16 (the remaining
# HT - FP8_K0 run as fp8e4 DoubleRow pairs at 2x).  2/8 bf16 keeps the
# quantization error at ~1.6e-2 of output scale vs the 2e-2 gate.
FP8_K0 = 2

NPBF16 = ml_dtypes.bfloat16


def _bcast_ap(src_row):
    """Partition-broadcast AP: read one [1, N] row as [128, N]."""
    return bass.AP(
        tensor=src_row.tensor,
        offset=src_row.offset,
        ap=[[0, P], list(src_row.ap[-1])],
    )


@with_exitstack
def _attn_kernel(
    ctx: ExitStack, tc: tile.TileContext, outs, ins, s=S, has_bq=False, has_bv=True
):
    nc = tc.nc
    JT = s // P  # key tiles along sequence
    NCH = s // CHUNK  # query chunks
    HT = H // P  # 8
    HC = H // CHUNK  # 2

    hsT = ins["hsT"].rearrange("(t p) s -> p t s", p=P)  # [128, HT, s]
    mT = ins["m"].rearrange("(t p) o -> p t o", p=P)  # [128, HT, H]
    wvT = ins["wvT"].rearrange("(t p) o -> p t o", p=P)
    relT = ins["relT"].rearrange("(t p) q -> p t q", p=P)  # [128, JT, s]
    ctxT = outs["ctxT"].rearrange("(t p) q -> p t q", p=P)  # [128, HT, s]

    consts = ctx.enter_context(tc.tile_pool(name="consts", bufs=1))
    qkv = ctx.enter_context(tc.tile_pool(name="qkv", bufs=1))

    # small per-partition constants
    m1_s = consts.tile([P, HT], BF16)
    nc.sync.dma_start(out=m1_s, in_=ins["m1"])
    am_s = consts.tile([P, JT], F32)
    nc.sync.dma_start(out=am_s, in_=ins["am"])
    if has_bv:
        # bv broadcast across partitions: [H] -> [128, H]
        bvB = consts.tile([P, H], BF16)
        nc.sync.dma_start(out=bvB, in_=_bcast_ap(ins["bv"]))
    ones_bf = consts.tile([P, 1], BF16)
    nc.vector.memset(ones_bf, 1.0)
    if has_bq:
        mb_s = consts.tile([P, HT], BF16)
        nc.sync.dma_start(out=mb_s, in_=ins["mb"])
        c_s = consts.tile([1, 1], F32)
        nc.sync.dma_start(out=c_s, in_=ins["c"])

    # persistent operands for the attention stage.  hs_sb stays resident:
    # it is the moving operand of G/w1 and the stationary of scores and
    # the V projection.  The scores contraction is mixed-precision: the
    # first FP8_K0 k-tiles in bf16 (GTb/hs_sb), the rest as fp8e4
    # DoubleRow pairs (G8/hs8).
    NF8 = HT - FP8_K0  # fp8 k-tiles
    hs_sb = qkv.tile([P, HT, s], BF16)
    GTb = qkv.tile([P, FP8_K0, s], BF16)
    G8 = qkv.tile([P, NF8, s], F8)
    hs8 = qkv.tile([P, NF8, s], F8)
    V = qkv.tile([P, JT, H], BF16)
    W1B = qkv.tile([P, NCH, CHUNK], BF16)  # (q . d1)/32, partition-broadcast

    # scores-phase PSUM pools opened BEFORE stage A's pool so they land in
    # banks stage A never touches — chunk-0 scores can start while stage A
    # epilogues drain.
    ps_score = ctx.enter_context(tc.tile_pool(name="ps_s", bufs=3, space="PSUM"))
    ps_vec = ctx.enter_context(tc.tile_pool(name="ps_vec", bufs=1, space="PSUM"))
    dram_scratch = ctx.enter_context(
        tc.tile_pool(name="dscratch", bufs=3 if has_bq else 2, space="DRAM")
    )
    # rel/bias pools outside the stage-A region so chunk-0's rel DMAs and
    # biasT writes don't WAR-wait on stage-A's hs/w tiles.
    relpool = ctx.enter_context(tc.tile_pool(name="rel", bufs=3))
    biaspool = ctx.enter_context(tc.tile_pool(name="bias", bufs=3))

    # ---- stage A: G projection, w1, V ----
    with (
        tc.tile_pool(name="stage_a", bufs=1) as sa,
        tc.tile_pool(name="psum_a", bufs=3, space="PSUM") as psa,
    ):
        m_sb = sa.tile([P, HT, H], BF16)
        wv_sb = sa.tile([P, HT, H], BF16)
        # Startup DMA waves sized/ordered so G's (c outer, ot inner) loop
        # consumes bytes in arrival order at the ~0.23 MB/us landing rate:
        # hs chunk 0 per-t (the it-accumulation of the first ot tracks the
        # stream), then M in ot-sized column blocks just ahead of each ot
        # pass, then the remaining hs chunks coarse, then wv.  Coarse late
        # waves keep the Sync issue queue (~0.7us/descriptor) short.
        nc.sync.dma_start(out=m_sb[:, :, 0:P], in_=mT[:, :, 0:P])
        for t in range(HT):
            nc.sync.dma_start(out=hs_sb[:, t, 0:CHUNK], in_=hsT[:, t, 0:CHUNK])
        for ob in range(1, HT):
            nc.sync.dma_start(
                out=m_sb[:, :, ob * P : (ob + 1) * P],
                in_=mT[:, :, ob * P : (ob + 1) * P],
            )
        for c in range(1, NCH):
            nc.sync.dma_start(
                out=hs_sb[:, :, c * CHUNK : (c + 1) * CHUNK],
                in_=hsT[:, :, c * CHUNK : (c + 1) * CHUNK],
            )
        nc.sync.dma_start(out=wv_sb, in_=wvT)

        # Dummy matmuls: keep the PE busy (and the HAM clock-gate warm)
        # while the first m/hs tiles stream in. Zero data, never consumed.
        dummy_src = sa.tile([P, 256], BF16)
        nc.vector.memset(dummy_src, 0.0)
        dummy_ps = ps_vec.tile([P, 256], F32, tag="vec1", name="dummy_ps")
        for _ in range(NDUMMY):
            nc.tensor.matmul(
                dummy_ps,
                dummy_src[:, 0:P],
                dummy_src,
                start=True,
                stop=True,
                skip_group_check=True,
            )

        # GT[o, s] = sum_i M[i, o] * hsT[i, s].  c outer / ot inner: each
        # c-pass re-reads the already-resident M and consumes exactly one
        # fresh hs chunk, so the PE never outruns the startup DMA stream.
        for c in range(NCH):
            for ot in range(HT):
                ps_g = psa.tile([P, CHUNK], F32, tag="pa", name="pa_g")
                for it in range(HT):
                    nc.tensor.matmul(
                        ps_g,
                        m_sb[:, it, ot * P : (ot + 1) * P],
                        hs_sb[:, it, c * CHUNK : (c + 1) * CHUNK],
                        start=(it == 0),
                        stop=(it == HT - 1),
                    )
                tgt = (
                    GTb[:, ot, c * CHUNK : (c + 1) * CHUNK]
                    if ot < FP8_K0
                    else G8[:, ot - FP8_K0, c * CHUNK : (c + 1) * CHUNK]
                )
                nc.scalar.activation(tgt, ps_g, AF.Identity, bias=0.0)

        # W1B precompute: w1[q]/32 = hs[q] . m1/32, all chunks, col-tiled 4x.
        # Emitted right after G so the DRAM-broadcast roundtrip completes
        # long before chunk 0 needs it.
        w1p = ps_vec.tile([P, CHUNK], F32, tag="vec1")
        for it in range(HT):
            for c in range(NCH):
                nc.tensor.matmul(
                    w1p[32 * c : 32 * c + 1, :],
                    m1_s[:, it : it + 1],
                    hs_sb[:, it, c * CHUNK : (c + 1) * CHUNK],
                    start=(it == 0),
                    stop=(it == HT - 1),
                    tile_position=(0, 32 * c),
                    skip_group_check=True,
                )
        w1rows = consts.tile([1, NCH, CHUNK], BF16)
        w1d = dram_scratch.tile([1, NCH, CHUNK], BF16)
        for c in range(NCH):
            if has_bq:
                # w1_full/32 = hs.m1/32 + (bq.d1)/32
                nc.scalar.activation(
                    w1rows[:, c, :],
                    w1p[32 * c : 32 * c + 1, :],
                    AF.Identity,
                    bias=c_s[0:1, 0:1],
                )
            else:
                nc.vector.tensor_copy(w1rows[:, c, :], w1p[32 * c : 32 * c + 1, :])
            nc.sync.dma_start(out=w1d[:, c, :], in_=w1rows[:, c, :])
            nc.sync.dma_start(out=W1B[:, c, :], in_=_bcast_ap(w1d[:, c, :]))

        if has_bq:
            # kb[k]/32 = hs[k] . (Wk.T bq)/32, added to the per-key exp bias.
            kbp = ps_vec.tile([P, CHUNK], F32, tag="vec1")
            for it in range(HT):
                for c in range(NCH):
                    nc.tensor.matmul(
                        kbp[32 * c : 32 * c + 1, :],
                        mb_s[:, it : it + 1],
                        hs_sb[:, it, c * CHUNK : (c + 1) * CHUNK],
                        start=(it == 0),
                        stop=(it == HT - 1),
                        tile_position=(0, 32 * c),
                        skip_group_check=True,
                    )
            kbrow = consts.tile([1, NCH, CHUNK], F32)
            for c in range(NCH):
                nc.vector.tensor_copy(kbrow[:, c, :], kbp[32 * c : 32 * c + 1, :])
            kbd = dram_scratch.tile([1, NCH, CHUNK], F32)
            nc.sync.dma_start(out=kbd, in_=kbrow)
            kb_s = consts.tile([P, JT], F32)
            nc.sync.dma_start(
                out=kb_s,
                in_=bass.AP(tensor=kbd.tensor, offset=kbd.offset, ap=[[1, P], [P, JT]]),
            )
            am_eff = consts.tile([P, JT], F32)
            nc.vector.tensor_tensor(am_eff, am_s, kb_s, op=ALU.add)
            am_x = am_eff
        else:
            am_x = am_s

        # fp8 copies of the hs k-tiles used by the DoubleRow score matmuls
        # (scalar engine; overlaps the V matmuls below)
        for i in range(NF8):
            nc.scalar.activation(
                hs8[:, i, :], hs_sb[:, i + FP8_K0, :], AF.Identity, bias=0.0
            )

        # V[j, o] = sum_i hsT[i, j] * WvT[i, o] + bv[o].  V last: it has no
        # chunk-0 consumers until PV, so its matmuls give the scheduler PE
        # filler while chunk-0's softmax pipeline warms up.
        for jt in range(JT):
            pss = [
                psa.tile([P, CHUNK], F32, tag="pa", name=f"pav_{i}")
                for i in range(HC)
            ]
            for it in range(HT):
                for oc in range(HC):
                    nc.tensor.matmul(
                        pss[oc],
                        hs_sb[:, it, jt * P : (jt + 1) * P],
                        wv_sb[:, it, oc * CHUNK : (oc + 1) * CHUNK],
                        start=(it == 0),
                        stop=(it == HT - 1),
                    )
            for oc in range(HC):
                if has_bv:
                    nc.vector.tensor_tensor(
                        V[:, jt, oc * CHUNK : (oc + 1) * CHUNK],
                        pss[oc],
                        bvB[:, oc * CHUNK : (oc + 1) * CHUNK],
                        op=ALU.add,
                    )
                else:
                    # scalar-engine copy keeps the vector queue clear for
                    # chunk 0's softmax pipeline
                    nc.scalar.activation(
                        V[:, jt, oc * CHUNK : (oc + 1) * CHUNK],
                        pss[oc],
                        AF.Identity,
                        bias=0.0,
                    )

    # ---- stage B pools ----
    epool = ctx.enter_context(tc.tile_pool(name="E", bufs=3))
    recpool = ctx.enter_context(tc.tile_pool(name="rec", bufs=2))
    outpool = ctx.enter_context(tc.tile_pool(name="out", bufs=3))
    ps_pv = ctx.enter_context(tc.tile_pool(name="ps_pv", bufs=4, space="PSUM"))

    # ---- stage B: per query chunk ----
    # PV for chunk c is emitted after the scores/softmax of chunk c+1, so
    # the denom->reciprocal->broadcast chain of chunk c overlaps an entire
    # scores phase and the PE never waits on it.
    deferred_pv = []

    def emit_pv(c, E, recB):
        cs = slice(c * CHUNK, (c + 1) * CHUNK)
        for dt in range(HT):
            ps2 = ps_pv.tile([P, CHUNK], F32)
            for jt in range(JT):
                nc.tensor.matmul(
                    ps2,
                    V[:, jt, dt * P : (dt + 1) * P],
                    E[:, jt, :],
                    start=(jt == 0),
                    stop=(jt == JT - 1),
                )
            ot_t = outpool.tile([P, CHUNK], F32)
            nc.vector.tensor_tensor(ot_t, ps2, recB, op=ALU.mult)
            nc.sync.dma_start(out=ctxT[:, dt, cs], in_=ot_t)

    for c in range(NCH):
        cs = slice(c * CHUNK, (c + 1) * CHUNK)

        E = epool.tile([P, JT, CHUNK], BF16)
        dps = ps_vec.tile([P, CHUNK], F32, tag="vec1")

        def denom_quad(jt0):
            # four adjacent col-tiled [128,1] matmuls run concurrently on
            # distinct PE column groups — ~4x the per-slot throughput
            for i, jj in enumerate(range(jt0, jt0 + 4)):
                nc.tensor.matmul(
                    dps[32 * i : 32 * i + 1, :],
                    ones_bf,
                    E[:, jj, :],
                    start=(jj < 4),
                    stop=(jj >= JT - 4),
                    tile_position=(0, 32 * i),
                    skip_group_check=True,
                )

        for jt in range(JT):
            rel_t = relpool.tile([P, CHUNK], I32)
            nc.sync.dma_start(out=rel_t, in_=relT[:, jt, cs])
            biasT = biaspool.tile([P, CHUNK], BF16)
            nc.vector.scalar_tensor_tensor(
                biasT, rel_t, 1, W1B[:, c, :], op0=ALU.is_equal, op1=ALU.mult
            )
            ps = ps_score.tile([P, CHUNK], F32)
            # Interleave DR pairs with the bf16 tiles: DoubleRow disables
            # FWL, so a DR LDWEIGHTS (~190ns) hides fully only under the
            # preceding matmul's stream — alternating gives every weight
            # load a full-length stream to hide under.
            seq = []
            for i in range(0, NF8, 2):
                seq.append(("dr", i))
                if i // 2 < FP8_K0:
                    seq.append(("bf", i // 2))
            for n, (kind, i) in enumerate(seq):
                first, last = n == 0, n == len(seq) - 1
                if kind == "dr":
                    nc.tensor.matmul(
                        ps,
                        hs8[:, i : i + 2, jt * P : (jt + 1) * P],
                        G8[:, i : i + 2, cs],
                        start=first,
                        stop=last,
                        perf_mode=DR,
                    )
                else:
                    nc.tensor.matmul(
                        ps,
                        hs_sb[:, i, jt * P : (jt + 1) * P],
                        GTb[:, i, cs],
                        start=first,
                        stop=last,
                    )
            nc.vector.scalar_tensor_tensor(
                ps, ps, SCALE, biasT, op0=ALU.mult, op1=ALU.add
            )
            nc.scalar.activation(E[:, jt, :], ps, AF.Exp, bias=am_x[:, jt : jt + 1])
            # interleave denominator accumulation a few tiles behind
            if jt >= 7 and jt % 4 == 3:
                denom_quad(jt - 7)

        # The final quad needs the last exp of this chunk (~1.5us behind
        # the PE) — emit the previous chunk's PV first so its matmuls fill
        # that wait instead of head-of-line blocking the PE queue.
        if deferred_pv:
            emit_pv(*deferred_pv.pop(0))
        denom_quad(JT - 4)

        # denom rows 0/32/64/96 -> sum -> broadcast -> reciprocal
        r32 = recpool.tile([1, CHUNK], F32, tag="r32")
        nc.vector.tensor_copy(r32, dps[32:33, :])
        r64 = recpool.tile([1, CHUNK], F32, tag="r64")
        nc.vector.tensor_copy(r64, dps[64:65, :])
        r96 = recpool.tile([1, CHUNK], F32, tag="r96")
        nc.vector.tensor_copy(r96, dps[96:97, :])
        s01 = recpool.tile([1, CHUNK], F32, tag="s01")
        nc.vector.tensor_tensor(s01, dps[0:1, :], r32, op=ALU.add)
        s23 = recpool.tile([1, CHUNK], F32, tag="s23")
        nc.vector.tensor_tensor(s23, r64, r96, op=ALU.add)
        dsum = recpool.tile([1, CHUNK], F32, tag="dsum")
        nc.vector.tensor_tensor(dsum, s01, s23, op=ALU.add)
        dsum_d = dram_scratch.tile([1, CHUNK], F32, tag="dsum_d")
        nc.sync.dma_start(out=dsum_d, in_=dsum)
        denB = recpool.tile([P, CHUNK], F32, tag="denB")
        nc.sync.dma_start(out=denB, in_=_bcast_ap(dsum_d))
        recB = recpool.tile([P, CHUNK], F32, tag="recB")
        rscr = recpool.tile([P, CHUNK], F32, tag="rscr")
        nc.vector.reciprocal_approx_accurate(recB, denB, rscr)

        deferred_pv.append((c, E, recB))
    while deferred_pv:
        emit_pv(*deferred_pv.pop(0))


def build_program(s=S, has_bq=False, has_bv=True):
    """Build + compile the per-core Bass program."""
    JT = s // P
    HT = H // P
    nc = bacc.Bacc("TRN2", target_bir_lowering=False, debug=False)
    ins = {
        "hsT": nc.dram_tensor("hsT", [H, s], BF16, kind="ExternalInput").ap(),
        "m": nc.dram_tensor("m", [H, H], BF16, kind="ExternalInput").ap(),
        "wvT": nc.dram_tensor("wvT", [H, H], BF16, kind="ExternalInput").ap(),
        "m1": nc.dram_tensor("m1", [P, HT], BF16, kind="ExternalInput").ap(),
        "am": nc.dram_tensor("am", [P, JT], F32, kind="ExternalInput").ap(),
        "relT": nc.dram_tensor("relT", [s, s], I32, kind="ExternalInput").ap(),
    }
    if has_bv:
        ins["bv"] = nc.dram_tensor("bv", [1, H], BF16, kind="ExternalInput").ap()
    if has_bq:
        ins["mb"] = nc.dram_tensor("mb", [P, HT], BF16, kind="ExternalInput").ap()
        ins["c"] = nc.dram_tensor("c", [1, 1], F32, kind="ExternalInput").ap()
    outs = {
        "ctxT": nc.dram_tensor("ctxT", [H, s], F32, kind="ExternalOutput").ap(),
    }
    with tile.TileContext(nc) as tc:
        _attn_kernel(tc, outs, ins, s=s, has_bq=has_bq, has_bv=has_bv)
    nc.compile()
    return nc


def make_in_maps(
    hidden_states,
    attention_mask,
    word_word_relation,
    Wq,
    bq,
    Wk,
    bk,
    Wv,
    bv,
    dist_emb,
    s=S,
):
    """Host-side sharding/layout marshalling: one batch per core.

    Weight-only folds (O(H^2), batch-independent): M = Wq.T @ Wk,
    m1 = Wq.T @ dist_emb[1].  bk only enters softmax-invariant terms.
    """
    HT = H // P
    JT = s // P
    hs = np.asarray(hidden_states, dtype=np.float32)
    am = np.asarray(attention_mask, dtype=np.float32)
    rel = np.ascontiguousarray(np.asarray(word_word_relation, dtype=np.int32))
    Wqf = np.asarray(Wq, np.float32)
    Wkf = np.asarray(Wk, np.float32)
    Wvf = np.asarray(Wv, np.float32)
    d1 = np.asarray(dist_emb, np.float32)[1]
    m_h = np.ascontiguousarray((Wqf.T @ Wkf).astype(NPBF16))
    m1_h = np.ascontiguousarray(
        ((Wqf.T @ d1) * SCALE).reshape(HT, P).T.astype(NPBF16)
    )
    wvT = np.ascontiguousarray(Wvf.T.astype(NPBF16))
    bvf = np.asarray(bv, np.float32)
    has_bv = bool(np.any(bvf))
    if has_bv:
        bv_s = np.ascontiguousarray(bvf.astype(NPBF16).reshape(1, H))
    bqf = np.asarray(bq, np.float32)
    has_bq = bool(np.any(bqf))
    if has_bq:
        mb_h = np.ascontiguousarray(
            ((Wkf.T @ bqf) * SCALE).reshape(HT, P).T.astype(NPBF16)
        )
        c_h = np.ascontiguousarray(
            np.array([[float(bqf @ d1) * SCALE]], dtype=np.float32)
        )
    in_maps = []
    for b in range(hs.shape[0]):
        hsT = np.ascontiguousarray(hs[b].T.astype(NPBF16))
        relT = np.ascontiguousarray(rel[b].T)
        am_s = np.ascontiguousarray(am[b, 0, 0].reshape(JT, P).T)
        im = {
            "hsT": hsT,
            "m": m_h,
            "wvT": wvT,
            "m1": m1_h,
            "am": am_s,
            "relT": relT,
        }
        if has_bv:
            im["bv"] = bv_s
        if has_bq:
            im["mb"] = mb_h
            im["c"] = c_h
        in_maps.append(im)
    return in_maps, has_bq, has_bv


_NC_CACHE = {}


def get_program(s=S, has_bq=False, has_bv=False):
    key = (s, has_bq, has_bv)
    if key not in _NC_CACHE:
        _NC_CACHE[key] = build_program(s, has_bq, has_bv)
    return _NC_CACHE[key]


def run(inputs: dict, trace: bool = False):
    """Run on hardware; returns (output [B,S,H] f32, BassKernelResults)."""
    in_maps, has_bq, has_bv = make_in_maps(**inputs)
    nc = get_program(S, has_bq, has_bv)
    res = run_bass_kernel_spmd(nc, in_maps, list(range(NCORES)), trace=trace)
    out = np.stack(
        [np.ascontiguousarray(r["ctxT"].T) for r in res.results], axis=0
    ).astype(np.float32)
    return out, res


def kernel(**inputs) -> np.ndarray:
    try:
        out, _ = run(inputs, trace=False)
    except Exception:
        # transient device/runtime hiccups have been observed once in a
        # while on back-to-back runs; one retry is cheap insurance
        out, _ = run(inputs, trace=False)
    return out


# revision 28
# speedup vs baseline: 1.0062x; 1.0062x over previous
"""Trainium2 Bass kernel for BertSelfAttentionDistance.

Problem (per batch b, B=8, S=2048, H=1024, NH=1, DT=64):
    q = hs @ Wq.T + bq ; k = hs @ Wk.T + bk ; v = hs @ Wv.T + bv
    scores = q @ k.T
    wdd    = q @ dist_emb.T                       # [S, DT]
    bias   = take_along(wdd, rel, axis=-1) * (rel == 1)
    out    = softmax((scores + bias)/sqrt(H) + attn_mask) @ v

Key simplifications:
1. Because the gathered value is multiplied by (rel == 1), only
   wdd[:, 1] = q @ dist_emb[1] survives:
       bias[i, j] = (rel[i, j] == 1) * (q[i] . dist_emb[1])
   so the [S, S] gather is never needed — just a compare and broadcast.
2. q and k are never needed individually — only scores and w1:
       scores0 = hs @ (Wq.T @ Wk) @ hs.T = G @ hs.T,   G = hs @ M
       w1      = hs @ (Wq.T @ dist_emb[1]) (+ bq . dist_emb[1])
   M = Wq.T @ Wk and m1 = Wq.T @ d1 are folded on the host (weight-only,
   O(H^2) preprocessing); this removes one full [S,H]x[H,H] projection
   per core and the hs tile doubles as the scores stationary.
   Bias terms: q0.bk and bq.bk are constant per query row -> softmax-
   invariant, dropped exactly. bq.k0[k] varies per key: handled by a
   conditionally-compiled kb path (has_bq) feeding the exp bias; the
   benchmark has bq = 0 so the lean variant is used.

Sharding: pure data-parallel, one batch per NeuronCore (8 batches, 8 cores).

Per-core layout (ST = transposed-scores layout, keys on partitions):
    GT[d, s]  (via M stationary / hsT moving; split GTb bf16 / G8 fp8e4)
    ST[j, q] = sum_d hsT[d, j] * GT[d, q]         (keys j on partitions)
    E[j, q]  = exp(ST/32 + (relT==1)*w1[q]/32 + am[j])   (unnormalized)
    denom[q] = ones[1,j] @ E    (PE reduction over partitions, 4x col-tiled)
    ctxT[d, q] = (sum_j (V[j, d]+bv[d]) * E[j, q]) * (1/denom[q])

Mixed precision: the scores contraction runs 2 of 8 k-tiles in bf16 and
6 as fp8e4 DoubleRow pairs (2x PE throughput).  Measured end-to-end
max-rel error 1.63e-2 of output scale (deterministic for the benchmark
inputs) vs the 2e-2 gate; all other matmuls stay bf16 (fp8 anywhere
else pushes past the gate - each further tensor quantization adds
~2e-2 in quadrature).

v bias note: softmax rows sum to 1, so ctx = P @ (V0 + 1*bv) = P@V0 + bv;
with unnormalized E: (E@(V0+1*bv)) / denom = ctx0 + bv exactly.
"""

import sys

sys.path.insert(0, "/opt/trn_rl_repo")

from contextlib import ExitStack

import ml_dtypes
import numpy as np

import concourse.bass as bass
import concourse.tile as tile
from concourse import bacc, mybir
from concourse._compat import with_exitstack
from concourse.bass_utils import run_bass_kernel_spmd

B, S, H, DT = 8, 2048, 1024, 64
NCORES = 8
P = 128
CHUNK = 512  # q-chunk = one fp32 PSUM bank
SCALE = 1.0 / 32.0  # 1/sqrt(H)
NDUMMY = 26

BF16 = mybir.dt.bfloat16
F8 = mybir.dt.float8e4
F32 = mybir.dt.float32
I32 = mybir.dt.int32
AF = mybir.ActivationFunctionType
ALU = mybir.AluOpType
DR = mybir.MatmulPerfMode.DoubleRow
# k-tiles of the scores contraction computed in bf16 (the remaining
# HT - FP8_K0 run as fp8e4 DoubleRow pairs at 2x).  2/8 bf16 keeps the
# quantization error at ~1.6e-2 of output scale vs the 2e-2 gate.
FP8_K0 = 2

NPBF16 = ml_dtypes.bfloat16


def _bcast_ap(src_row):
    """Partition-broadcast AP: read one [1, N] row as [128, N]."""
    return bass.AP(
        tensor=src_row.tensor,
        offset=src_row.offset,
        ap=[[0, P], list(src_row.ap[-1])],
    )


@with_exitstack
def _attn_kernel(
    ctx: ExitStack, tc: tile.TileContext, outs, ins, s=S, has_bq=False, has_bv=True
):
    nc = tc.nc
    JT = s // P  # key tiles along sequence
    NCH = s // CHUNK  # query chunks
    HT = H // P  # 8
    HC = H // CHUNK  # 2

    hsT = ins["hsT"].rearrange("(t p) s -> p t s", p=P)  # [128, HT, s]
    mT = ins["m"].rearrange("(t p) o -> p t o", p=P)  # [128, HT, H]
    wvT = ins["wvT"].rearrange("(t p) o -> p t o", p=P)
    relT = ins["relT"].rearrange("(t p) q -> p t q", p=P)  # [128, JT, s]
    ctxT = outs["ctxT"].rearrange("(t p) q -> p t q", p=P)  # [128, HT, s]

    consts = ctx.enter_context(tc.tile_pool(name="consts", bufs=1))
    qkv = ctx.enter_context(tc.tile_pool(name="qkv", bufs=1))

    # small per-partition constants
    m1_s = consts.tile([P, HT], BF16)
    nc.sync.dma_start(out=m1_s, in_=ins["m1"])
    am_s = consts.tile([P, JT], F32)
    nc.sync.dma_start(out=am_s, in_=ins["am"])
    if has_bv:
        # bv broadcast across partitions: [H] -> [128, H]
        bvB = consts.tile([P, H], BF16)
        nc.sync.dma_start(out=bvB, in_=_bcast_ap(ins["bv"]))
    ones_bf = consts.tile([P, 1], BF16)
    nc.vector.memset(ones_bf, 1.0)
    if has_bq:
        mb_s = consts.tile([P, HT], BF16)
        nc.sync.dma_start(out=mb_s, in_=ins["mb"])
        c_s = consts.tile([1, 1], F32)
        nc.sync.dma_start(out=c_s, in_=ins["c"])

    # persistent operands for the attention stage.  hs_sb stays resident:
    # it is the moving operand of G/w1 and the stationary of scores and
    # the V projection.  The scores contraction is mixed-precision: the
    # first FP8_K0 k-tiles in bf16 (GTb/hs_sb), the rest as fp8e4
    # DoubleRow pairs (G8/hs8).
    NF8 = HT - FP8_K0  # fp8 k-tiles
    hs_sb = qkv.tile([P, HT, s], BF16)
    GTb = qkv.tile([P, FP8_K0, s], BF16)
    G8 = qkv.tile([P, NF8, s], F8)
    hs8 = qkv.tile([P, NF8, s], F8)
    V = qkv.tile([P, JT, H], BF16)
    W1B = qkv.tile([P, NCH, CHUNK], BF16)  # (q . d1)/32, partition-broadcast

    # scores-phase PSUM pools opened BEFORE stage A's pool so they land in
    # banks stage A never touches — chunk-0 scores can start while stage A
    # epilogues drain.
    ps_score = ctx.enter_context(tc.tile_pool(name="ps_s", bufs=3, space="PSUM"))
    ps_vec = ctx.enter_context(tc.tile_pool(name="ps_vec", bufs=1, space="PSUM"))
    dram_scratch = ctx.enter_context(
        tc.tile_pool(name="dscratch", bufs=3 if has_bq else 2, space="DRAM")
    )
    # rel/bias pools outside the stage-A region so chunk-0's rel DMAs and
    # biasT writes don't WAR-wait on stage-A's hs/w tiles.
    relpool = ctx.enter_context(tc.tile_pool(name="rel", bufs=3))
    biaspool = ctx.enter_context(tc.tile_pool(name="bias", bufs=3))

    # ---- stage A: G projection, w1, V ----
    with (
        tc.tile_pool(name="stage_a", bufs=1) as sa,
        tc.tile_pool(name="psum_a", bufs=3, space="PSUM") as psa,
    ):
        m_sb = sa.tile([P, HT, H], BF16)
        wv_sb = sa.tile([P, HT, H], BF16)
        # Startup DMA waves sized/ordered so G's (c outer, ot inner) loop
        # consumes bytes in arrival order at the ~0.23 MB/us landing rate:
        # hs chunk 0 per-t (the it-accumulation of the first ot tracks the
        # stream), then M in ot-sized column blocks just ahead of each ot
        # pass, then the remaining hs chunks coarse, then wv.  Coarse late
        # waves keep the Sync issue queue (~0.7us/descriptor) short.
        nc.sync.dma_start(out=m_sb[:, :, 0:P], in_=mT[:, :, 0:P])
        for t in range(HT):
            nc.sync.dma_start(out=hs_sb[:, t, 0:CHUNK], in_=hsT[:, t, 0:CHUNK])
        for ob in range(1, HT):
            nc.sync.dma_start(
                out=m_sb[:, :, ob * P : (ob + 1) * P],
                in_=mT[:, :, ob * P : (ob + 1) * P],
            )
        for c in range(1, NCH):
            nc.sync.dma_start(
                out=hs_sb[:, :, c * CHUNK : (c + 1) * CHUNK],
                in_=hsT[:, :, c * CHUNK : (c + 1) * CHUNK],
            )
        nc.sync.dma_start(out=wv_sb, in_=wvT)

        # Dummy matmuls: keep the PE busy (and the HAM clock-gate warm)
        # while the first m/hs tiles stream in. Zero data, never consumed.
        dummy_src = sa.tile([P, 256], BF16)
        nc.vector.memset(dummy_src, 0.0)
        dummy_ps = ps_vec.tile([P, 256], F32, tag="vec1", name="dummy_ps")
        for _ in range(NDUMMY):
            nc.tensor.matmul(
                dummy_ps,
                dummy_src[:, 0:P],
                dummy_src,
                start=True,
                stop=True,
                skip_group_check=True,
            )

        # GT[o, s] = sum_i M[i, o] * hsT[i, s].  c outer / ot inner: each
        # c-pass re-reads the already-resident M and consumes exactly one
        # fresh hs chunk, so the PE never outruns the startup DMA stream.
        for c in range(NCH):
            for ot in range(HT):
                ps_g = psa.tile([P, CHUNK], F32, tag="pa", name="pa_g")
                for it in range(HT):
                    nc.tensor.matmul(
                        ps_g,
                        m_sb[:, it, ot * P : (ot + 1) * P],
                        hs_sb[:, it, c * CHUNK : (c + 1) * CHUNK],
                        start=(it == 0),
                        stop=(it == HT - 1),
                    )
                tgt = (
                    GTb[:, ot, c * CHUNK : (c + 1) * CHUNK]
                    if ot < FP8_K0
                    else G8[:, ot - FP8_K0, c * CHUNK : (c + 1) * CHUNK]
                )
                nc.scalar.activation(tgt, ps_g, AF.Identity, bias=0.0)

        # W1B precompute: w1[q]/32 = hs[q] . m1/32, all chunks, col-tiled 4x.
        # Emitted right after G so the DRAM-broadcast roundtrip completes
        # long before chunk 0 needs it.
        w1p = ps_vec.tile([P, CHUNK], F32, tag="vec1")
        for it in range(HT):
            for c in range(NCH):
                nc.tensor.matmul(
                    w1p[32 * c : 32 * c + 1, :],
                    m1_s[:, it : it + 1],
                    hs_sb[:, it, c * CHUNK : (c + 1) * CHUNK],
                    start=(it == 0),
                    stop=(it == HT - 1),
                    tile_position=(0, 32 * c),
                    skip_group_check=True,
                )
        w1rows = consts.tile([1, NCH, CHUNK], BF16)
        w1d = dram_scratch.tile([1, NCH, CHUNK], BF16)
        for c in range(NCH):
            if has_bq:
                # w1_full/32 = hs.m1/32 + (bq.d1)/32
                nc.scalar.activation(
                    w1rows[:, c, :],
                    w1p[32 * c : 32 * c + 1, :],
                    AF.Identity,
                    bias=c_s[0:1, 0:1],
                )
            else:
                nc.vector.tensor_copy(w1rows[:, c, :], w1p[32 * c : 32 * c + 1, :])
            nc.sync.dma_start(out=w1d[:, c, :], in_=w1rows[:, c, :])
            nc.sync.dma_start(out=W1B[:, c, :], in_=_bcast_ap(w1d[:, c, :]))

        if has_bq:
            # kb[k]/32 = hs[k] . (Wk.T bq)/32, added to the per-key exp bias.
            kbp = ps_vec.tile([P, CHUNK], F32, tag="vec1")
            for it in range(HT):
                for c in range(NCH):
                    nc.tensor.matmul(
                        kbp[32 * c : 32 * c + 1, :],
                        mb_s[:, it : it + 1],
                        hs_sb[:, it, c * CHUNK : (c + 1) * CHUNK],
                        start=(it == 0),
                        stop=(it == HT - 1),
                        tile_position=(0, 32 * c),
                        skip_group_check=True,
                    )
            kbrow = consts.tile([1, NCH, CHUNK], F32)
            for c in range(NCH):
                nc.vector.tensor_copy(kbrow[:, c, :], kbp[32 * c : 32 * c + 1, :])
            kbd = dram_scratch.tile([1, NCH, CHUNK], F32)
            nc.sync.dma_start(out=kbd, in_=kbrow)
            kb_s = consts.tile([P, JT], F32)
            nc.sync.dma_start(
                out=kb_s,
                in_=bass.AP(tensor=kbd.tensor, offset=kbd.offset, ap=[[1, P], [P, JT]]),
            )
            am_eff = consts.tile([P, JT], F32)
            nc.vector.tensor_tensor(am_eff, am_s, kb_s, op=ALU.add)
            am_x = am_eff
        else:
            am_x = am_s

        # fp8 copies of the hs k-tiles used by the DoubleRow score matmuls
        # (scalar engine; overlaps the V matmuls below)
        for i in range(NF8):
            nc.scalar.activation(
                hs8[:, i, :], hs_sb[:, i + FP8_K0, :], AF.Identity, bias=0.0
            )

        # V[j, o] = sum_i hsT[i, j] * WvT[i, o] + bv[o].  V last: it has no
        # chunk-0 consumers until PV, so its matmuls give the scheduler PE
        # filler while chunk-0's softmax pipeline warms up.
        for jt in range(JT):
            pss = [
                psa.tile([P, CHUNK], F32, tag="pa", name=f"pav_{i}")
                for i in range(HC)
            ]
            for it in range(HT):
                for oc in range(HC):
                    nc.tensor.matmul(
                        pss[oc],
                        hs_sb[:, it, jt * P : (jt + 1) * P],
                        wv_sb[:, it, oc * CHUNK : (oc + 1) * CHUNK],
                        start=(it == 0),
                        stop=(it == HT - 1),
                    )
            for oc in range(HC):
                if has_bv:
                    nc.vector.tensor_tensor(
                        V[:, jt, oc * CHUNK : (oc + 1) * CHUNK],
                        pss[oc],
                        bvB[:, oc * CHUNK : (oc + 1) * CHUNK],
                        op=ALU.add,
                    )
                else:
                    # scalar-engine copy keeps the vector queue clear for
                    # chunk 0's softmax pipeline
                    nc.scalar.activation(
                        V[:, jt, oc * CHUNK : (oc + 1) * CHUNK],
                        pss[oc],
                        AF.Identity,
                        bias=0.0,
                    )

    # ---- stage B pools ----
    epool = ctx.enter_context(tc.tile_pool(name="E", bufs=3))
    recpool = ctx.enter_context(tc.tile_pool(name="rec", bufs=2))
    outpool = ctx.enter_context(tc.tile_pool(name="out", bufs=3))
    ps_pv = ctx.enter_context(tc.tile_pool(name="ps_pv", bufs=4, space="PSUM"))

    # ---- stage B: per query chunk ----
    # PV for chunk c is emitted after the scores/softmax of chunk c+1, so
    # the denom->reciprocal->broadcast chain of chunk c overlaps an entire
    # scores phase and the PE never waits on it.
    deferred_pv = []

    def emit_pv(c, E, recB):
        cs = slice(c * CHUNK, (c + 1) * CHUNK)
        for dt in range(HT):
            ps2 = ps_pv.tile([P, CHUNK], F32)
            for jt in range(JT):
                nc.tensor.matmul(
                    ps2,
                    V[:, jt, dt * P : (dt + 1) * P],
                    E[:, jt, :],
                    start=(jt == 0),
                    stop=(jt == JT - 1),
                )
            ot_t = outpool.tile([P, CHUNK], F32)
            nc.vector.tensor_tensor(ot_t, ps2, recB, op=ALU.mult)
            nc.sync.dma_start(out=ctxT[:, dt, cs], in_=ot_t)

    for c in range(NCH):
        cs = slice(c * CHUNK, (c + 1) * CHUNK)

        E = epool.tile([P, JT, CHUNK], BF16)
        dps = ps_vec.tile([P, CHUNK], F32, tag="vec1")

        def denom_quad(jt0):
            # four adjacent col-tiled [128,1] matmuls run concurrently on
            # distinct PE column groups — ~4x the per-slot throughput
            for i, jj in enumerate(range(jt0, jt0 + 4)):
                nc.tensor.matmul(
                    dps[32 * i : 32 * i + 1, :],
                    ones_bf,
                    E[:, jj, :],
                    start=(jj < 4),
                    stop=(jj >= JT - 4),
                    tile_position=(0, 32 * i),
                    skip_group_check=True,
                )

        for jt in range(JT):
            rel_t = relpool.tile([P, CHUNK], I32)
            nc.sync.dma_start(out=rel_t, in_=relT[:, jt, cs])
            biasT = biaspool.tile([P, CHUNK], BF16)
            nc.vector.scalar_tensor_tensor(
                biasT, rel_t, 1, W1B[:, c, :], op0=ALU.is_equal, op1=ALU.mult
            )
            ps = ps_score.tile([P, CHUNK], F32)
            for dt in range(FP8_K0):
                nc.tensor.matmul(
                    ps,
                    hs_sb[:, dt, jt * P : (jt + 1) * P],
                    GTb[:, dt, cs],
                    start=(dt == 0),
                    stop=False,
                )
            for i in range(0, NF8, 2):
                nc.tensor.matmul(
                    ps,
                    hs8[:, i : i + 2, jt * P : (jt + 1) * P],
                    G8[:, i : i + 2, cs],
                    start=False,
                    stop=(i == NF8 - 2),
                    perf_mode=DR,
                )
            nc.vector.scalar_tensor_tensor(
                ps, ps, SCALE, biasT, op0=ALU.mult, op1=ALU.add
            )
            nc.scalar.activation(E[:, jt, :], ps, AF.Exp, bias=am_x[:, jt : jt + 1])
            # interleave denominator accumulation a few tiles behind
            if jt >= 7 and jt % 4 == 3:
                denom_quad(jt - 7)

        # The final quad needs the last exp of this chunk (~1.5us behind
        # the PE) — emit the previous chunk's PV first so its matmuls fill
        # that wait instead of head-of-line blocking the PE queue.
        if deferred_pv:
            emit_pv(*deferred_pv.pop(0))
        denom_quad(JT - 4)

        # denom rows 0/32/64/96 -> sum -> broadcast -> reciprocal
        r32 = recpool.tile([1, CHUNK], F32, tag="r32")
        nc.vector.tensor_copy(r32, dps[32:33, :])
        r64 = recpool.tile([1, CHUNK], F32, tag="r64")
        nc.vector.tensor_copy(r64, dps[64:65, :])
        r96 = recpool.tile([1, CHUNK], F32, tag="r96")
        nc.vector.tensor_copy(r96, dps[96:97, :])
        s01 = recpool.tile([1, CHUNK], F32, tag="s01")
        nc.vector.tensor_tensor(s01, dps[0:1, :], r32, op=ALU.add)
        s23 = recpool.tile([1, CHUNK], F32, tag="s23")
        nc.vector.tensor_tensor(s23, r64, r96, op=ALU.add)
        dsum = recpool.tile([1, CHUNK], F32, tag="dsum")
        nc.vector.tensor_tensor(dsum, s01, s23, op=ALU.add)
        dsum_d = dram_scratch.tile([1, CHUNK], F32, tag="dsum_d")
        nc.sync.dma_start(out=dsum_d, in_=dsum)
        denB = recpool.tile([P, CHUNK], F32, tag="denB")
        nc.sync.dma_start(out=denB, in_=_bcast_ap(dsum_d))
        recB = recpool.tile([P, CHUNK], F32, tag="recB")
        rscr = recpool.tile([P, CHUNK], F32, tag="rscr")
        nc.vector.reciprocal_approx_accurate(recB, denB, rscr)

        deferred_pv.append((c, E, recB))
    while deferred_pv:
        emit_pv(*deferred_pv.pop(0))


def build_program(s=S, has_bq=False, has_bv=True):
    """Build + compile the per-core Bass program."""
    JT = s // P
    HT = H // P
    nc = bacc.Bacc("TRN2", target_bir_lowering=False, debug=False)
    ins = {
        "hsT": nc.dram_tensor("hsT", [H, s], BF16, kind="ExternalInput").ap(),
        "m": nc.dram_tensor("m", [H, H], BF16, kind="ExternalInput").ap(),
        "wvT": nc.dram_tensor("wvT", [H, H], BF16, kind="ExternalInput").ap(),
        "m1": nc.dram_tensor("m1", [P, HT], BF16, kind="ExternalInput").ap(),
        "am": nc.dram_tensor("am", [P, JT], F32, kind="ExternalInput").ap(),
        "relT": nc.dram_tensor("relT", [s, s], I32, kind="ExternalInput").ap(),
    }
    if has_bv:
        ins["bv"] = nc.dram_tensor("bv", [1, H], BF16, kind="ExternalInput").ap()
    if has_bq:
        ins["mb"] = nc.dram_tensor("mb", [P, HT], BF16, kind="ExternalInput").ap()
        ins["c"] = nc.dram_tensor("c", [1, 1], F32, kind="ExternalInput").ap()
    outs = {
        "ctxT": nc.dram_tensor("ctxT", [H, s], F32, kind="ExternalOutput").ap(),
    }
    with tile.TileContext(nc) as tc:
        _attn_kernel(tc, outs, ins, s=s, has_bq=has_bq, has_bv=has_bv)
    nc.compile()
    return nc


def make_in_maps(
    hidden_states,
    attention_mask,
    word_word_relation,
    Wq,
    bq,
    Wk,
    bk,
    Wv,
    bv,
    dist_emb,
    s=S,
):
    """Host-side sharding/layout marshalling: one batch per core.

    Weight-only folds (O(H^2), batch-independent): M = Wq.T @ Wk,
    m1 = Wq.T @ dist_emb[1].  bk only enters softmax-invariant terms.
    """
    HT = H // P
    JT = s // P
    hs = np.asarray(hidden_states, dtype=np.float32)
    am = np.asarray(attention_mask, dtype=np.float32)
    rel = np.ascontiguousarray(np.asarray(word_word_relation, dtype=np.int32))
    Wqf = np.asarray(Wq, np.float32)
    Wkf = np.asarray(Wk, np.float32)
    Wvf = np.asarray(Wv, np.float32)
    d1 = np.asarray(dist_emb, np.float32)[1]
    m_h = np.ascontiguousarray((Wqf.T @ Wkf).astype(NPBF16))
    m1_h = np.ascontiguousarray(
        ((Wqf.T @ d1) * SCALE).reshape(HT, P).T.astype(NPBF16)
    )
    wvT = np.ascontiguousarray(Wvf.T.astype(NPBF16))
    bvf = np.asarray(bv, np.float32)
    has_bv = bool(np.any(bvf))
    if has_bv:
        bv_s = np.ascontiguousarray(bvf.astype(NPBF16).reshape(1, H))
    bqf = np.asarray(bq, np.float32)
    has_bq = bool(np.any(bqf))
    if has_bq:
        mb_h = np.ascontiguousarray(
            ((Wkf.T @ bqf) * SCALE).reshape(HT, P).T.astype(NPBF16)
        )
        c_h = np.ascontiguousarray(
            np.array([[float(bqf @ d1) * SCALE]], dtype=np.float32)
        )
    in_maps = []
    for b in range(hs.shape[0]):
        hsT = np.ascontiguousarray(hs[b].T.astype(NPBF16))
        relT = np.ascontiguousarray(rel[b].T)
        am_s = np.ascontiguousarray(am[b, 0, 0].reshape(JT, P).T)
        im = {
            "hsT": hsT,
            "m": m_h,
            "wvT": wvT,
            "m1": m1_h,
            "am": am_s,
            "relT": relT,
        }
        if has_bv:
            im["bv"] = bv_s
        if has_bq:
            im["mb"] = mb_h
            im["c"] = c_h
        in_maps.append(im)
    return in_maps, has_bq, has_bv


_NC_CACHE = {}


def get_program(s=S, has_bq=False, has_bv=False):
    key = (s, has_bq, has_bv)
    if key not in _NC_CACHE:
        _NC_CACHE[key] = build_program(s, has_bq, has_bv)
    return _NC_CACHE[key]


def run(inputs: dict, trace: bool = False):
    """Run on hardware; returns (output [B,S,H] f32, BassKernelResults)."""
    in_maps, has_bq, has_bv = make_in_maps(**inputs)
    nc = get_program(S, has_bq, has_bv)
    res = run_bass_kernel_spmd(nc, in_maps, list(range(NCORES)), trace=trace)
    out = np.stack(
        [np.ascontiguousarray(r["ctxT"].T) for r in res.results], axis=0
    ).astype(np.float32)
    return out, res


def kernel(**inputs) -> np.ndarray:
    try:
        out, _ = run(inputs, trace=False)
    except Exception:
        # transient device/runtime hiccups have been observed once in a
        # while on back-to-back runs; one retry is cheap insurance
        out, _ = run(inputs, trace=False)
    return out
